# revision 8
# baseline (speedup 1.0000x reference)
"""Trainium2 Bass kernel for per-pixel kernel application (KPN-style ApplyKernel).

y[c,h,w] = sum_{ii,jj} xpad[c, h+ii, w+jj] * k[ii*11+jj, h, w]

Strategy (8 NeuronCores, data-parallel over H strips of 90 rows):
  - Partition p owns a 10-column block of W (128 partitions x 10 = 1280), with
    the +-5 column halo stored in the free dim, so both row and column shifts
    of a tap are plain access-pattern offsets (DVE lanes are partition-locked,
    so shifts must live in the free dim). All 128 lanes are used.
  - Host: pad x and build bf16 slabs [128, 3ch x 100rows x 20cols] in two
    column-alignment variants so every tap's VectorE read stays 4-byte
    aligned, keeping tensor_tensor in its 2x bf16 mode. k is re-laid-out
    host-side to bf16 [128, 121, 900] (partition-block-major, even-column
    taps first) so HBM traffic is halved and DMA descriptors are large
    contiguous chunks.
  - Per tap: one elementwise multiply producing a bf16 [128, 2700] product
    tile, then 6 TensorE identity-matmuls (K=M=128) accumulating into 6 PSUM
    banks (3 channels x 512/388-col chunks).  The multiplies are split
    between VectorE (~97 taps, bf16 2x mode, 1.49us each) and GpSimd
    (~24 taps, 6.0us each) so that DVE, Pool and PE all run ~144us busy.
    GpSimd taps are issued LOOKAHEAD positions early so their products are
    ready when the in-order PE accumulation stream reaches them.
  - k groups are double-buffered (group g+1 DMA'd at the start of group g)
    on the ACT HWDGE ring; slabs/ident/y use the SP ring so the fill phase
    runs both rings in parallel.
  - Epilogue: ScalarE+VectorE evacuate PSUM -> SBUF bf16, per-channel DMAs
    out, host-side reshape + f32 cast of y.
"""

import sys

if "/opt/trn_rl_repo" not in sys.path:
    sys.path.insert(0, "/opt/trn_rl_repo")

import numpy as np
import ml_dtypes

import concourse.mybir as mybir
from concourse import bacc
from concourse.tile import TileContext, add_dep_helper
from concourse.bass_utils import run_bass_kernel_spmd

KS = 11
HALF = 5
H, W, C = 720, 1280, 3
NCORES = 8
HS = H // NCORES            # 90 rows per core
NP = 128                    # partitions (one 10-col block each)
CPP = W // NP               # 10 output cols per partition
ROWS_ST = HS + 2 * HALF     # 100 rows stored per partition
COLS_ST = CPP + 2 * HALF    # 20 cols stored per partition
SLABF = C * ROWS_ST * COLS_ST   # 6000 bf16 per partition per variant
NTAPS = KS * KS             # 121
FD = HS * CPP               # 900 elements per channel per tap
PFD = C * FD                # 2700 product elements per tap
N0, N1 = 512, FD - 512      # matmul chunk widths per channel (512, 388)

# tap order: even-jj taps first (only need slab variant 0), then odd-jj
TAP_PERM = ([t for t in range(NTAPS) if (t % KS) % 2 == 0]
            + [t for t in range(NTAPS) if (t % KS) % 2 == 1])
# k DMA group sizes over the permuted order; group g+1 is prefetched at the
# start of group g's processing, so only the first two need to be small.
GROUPS = [2, 3, 3, 5] + [8] * 13 + [4]
assert sum(GROUPS) == NTAPS

# taps whose multiply runs on GpSimd (Pool) instead of VectorE; ~1 in 5,
# spread evenly so the in-order PE stream never waits long on one engine
POOL_TAPS = frozenset(gi for gi in range(NTAPS) if gi % 5 == 2)
LOOKAHEAD = 4               # pool TT issued this many positions early

BF16 = ml_dtypes.bfloat16

_CACHE = {}


def _build_nc(taps=NTAPS):
    nc = bacc.Bacc("TRN2", target_bir_lowering=False, debug=False)
    k_d = nc.dram_tensor("k", [NP, NTAPS, FD], mybir.dt.bfloat16, kind="ExternalInput")
    xs_d = nc.dram_tensor("xs", [2, NP, SLABF], mybir.dt.bfloat16, kind="ExternalInput")
    id_d = nc.dram_tensor("ident", [NP, NP], mybir.dt.bfloat16, kind="ExternalInput")
    y_d = nc.dram_tensor("y", [NP, PFD], mybir.dt.bfloat16, kind="ExternalOutput")

    groups = []
    gi0 = 0
    for ng in GROUPS:
        if gi0 >= taps:
            break
        groups.append((gi0, min(ng, taps - gi0)))
        gi0 += ng
    grp_of = {}
    for g, (gi0, ng) in enumerate(groups):
        for dt_ in range(ng):
            grp_of[gi0 + dt_] = (g, dt_)

    pool_issue = {}         # pos -> [gi...] pool TTs to issue at this position
    for gi in range(taps):
        if gi in POOL_TAPS:
            pool_issue.setdefault(max(0, gi - LOOKAHEAD), []).append(gi)

    with TileContext(nc) as tc:
        with tc.tile_pool(name="const", bufs=1) as const_pool, \
             tc.tile_pool(name="kbf", bufs=4) as kb_pool, \
             tc.tile_pool(name="prod", bufs=8) as prod_pool, \
             tc.tile_pool(name="out", bufs=1) as out_pool, \
             tc.tile_pool(name="psum", bufs=1, space="PSUM") as psum_pool:

            slab0 = const_pool.tile([NP, SLABF], mybir.dt.bfloat16)
            slab1 = const_pool.tile([NP, SLABF], mybir.dt.bfloat16)
            ident = const_pool.tile([NP, NP], mybir.dt.bfloat16)

            kb_tiles = {}

            def dma_group(g):
                gi0, ng = groups[g]
                kb = kb_pool.tile([NP, ng * FD], mybir.dt.bfloat16, name="kb")
                nc.scalar.dma_start(
                    kb[:].rearrange("p (t f) -> p t f", t=ng),
                    k_d.ap()[:, gi0:gi0 + ng, :])
                kb_tiles[g] = kb

            # Fill phase: ident + first k group first (small, unblock the
            # start), slab halves split across both HWDGE rings.
            HL = SLABF // 2
            nc.sync.dma_start(ident[:], id_d.ap())
            dma_group(0)                                            # ACT ring
            nc.sync.dma_start(slab0[:, :HL], xs_d.ap()[0, :, :HL])  # SP ring
            nc.scalar.dma_start(slab0[:, HL:], xs_d.ap()[0, :, HL:])
            dma_group(1)
            nc.sync.dma_start(slab1[:, :HL], xs_d.ap()[1, :, :HL])
            nc.sync.dma_start(slab1[:, HL:], xs_d.ap()[1, :, HL:])

            slab_views = [
                s[:].rearrange("p (c r w) -> p c r w", c=C, r=ROWS_ST, w=COLS_ST)
                for s in (slab0, slab1)]

            accs = []
            for c in range(C):
                a0 = psum_pool.tile([NP, N0], mybir.dt.float32, name=f"acc{c}0")
                a1 = psum_pool.tile([NP, N1], mybir.dt.float32, name=f"acc{c}1")
                accs.append((a0, a1))

            prods = {}

            def issue_tt(gi, eng):
                g, dt_ = grp_of[gi]
                kb = kb_tiles[g]
                t = TAP_PERM[gi]
                ii, jj = divmod(t, KS)
                v = jj & 1
                jj2 = jj - v
                xs_op = slab_views[v][:, :, ii:ii + HS, jj2:jj2 + CPP]
                k_op = (kb[:, dt_ * FD:(dt_ + 1) * FD]
                        .rearrange("p (r w) -> p r w", r=HS)
                        .unsqueeze(1).broadcast_to([NP, C, HS, CPP]))
                prod = prod_pool.tile([NP, PFD], mybir.dt.bfloat16, name="prod")
                prod_view = prod[:].rearrange(
                    "p (c r w) -> p c r w", c=C, r=HS, w=CPP)
                eng.tensor_tensor(prod_view, xs_op, k_op, mybir.AluOpType.mult)
                prods[gi] = prod

            for pos in range(taps):
                g, dt_ = grp_of[pos]
                if dt_ == 0 and g + 1 < len(groups) and g >= 1:
                    dma_group(g + 1)
                for pgi in pool_issue.get(pos, []):
                    issue_tt(pgi, nc.gpsimd)
                if pos not in POOL_TAPS:
                    issue_tt(pos, nc.vector)
                prod = prods.pop(pos)
                first = (pos == 0)
                last = (pos == taps - 1)
                for c in range(C):
                    nc.tensor.matmul(accs[c][0][:], ident[:],
                                     prod[:, c * FD:c * FD + N0],
                                     start=first, stop=last)
                    nc.tensor.matmul(accs[c][1][:], ident[:],
                                     prod[:, c * FD + N0:(c + 1) * FD],
                                     start=first, stop=last)

            yst = out_pool.tile([NP, PFD], mybir.dt.bfloat16)
            for c in range(C):
                nc.scalar.copy(yst[:, c * FD:c * FD + N0], accs[c][0][:])
                nc.vector.tensor_copy(yst[:, c * FD + N0:(c + 1) * FD],
                                      accs[c][1][:])
                nc.sync.dma_start(y_d.ap()[:, c * FD:(c + 1) * FD],
                                  yst[:, c * FD:(c + 1) * FD])

    nc.compile()
    return nc


def get_nc(taps=NTAPS):
    if taps not in _CACHE:
        _CACHE[taps] = _build_nc(taps)
    return _CACHE[taps]


def _prep_inputs(x, k, padding, padding_value):
    """Host-side prep: pad x, build bf16 slabs + per-core shards."""
    x = np.asarray(x, dtype=np.float32)
    k = np.asarray(k, dtype=np.float32)
    pad = bool(int(np.asarray(padding)))
    pv = float(np.asarray(padding_value))

    if pad:
        assert x.shape == (1, C, H, W), x.shape
        xp = np.full((C, H + 2 * HALF, W + 2 * HALF + 1), 0.0, dtype=np.float32)
        xp[:, :, :W + 2 * HALF] = pv
        xp[:, HALF:HALF + H, HALF:HALF + W] = x[0]
    else:
        assert x.shape == (1, C, H + 2 * HALF, W + 2 * HALF), x.shape
        xp = np.zeros((C, H + 2 * HALF, W + 2 * HALF + 1), dtype=np.float32)
        xp[:, :, :W + 2 * HALF] = x[0]

    assert k.shape == (1, NTAPS, H, W), k.shape
    # partition-block-major, tap-permuted k: [core, p, t, (r w)], bf16
    kt_all = np.ascontiguousarray(
        k[0][TAP_PERM].astype(BF16).reshape(NTAPS, NCORES, HS, NP, CPP)
        .transpose(1, 3, 0, 2, 4)).reshape(NCORES, NP, NTAPS, FD)

    cols_idx = CPP * np.arange(NP)[:, None] + np.arange(COLS_ST)[None, :]
    ident = np.eye(NP, dtype=BF16)
    in_maps = []
    for ci in range(NCORES):
        rows = slice(HS * ci, HS * ci + ROWS_ST)
        xs = np.empty((2, NP, SLABF), dtype=BF16)
        for v in (0, 1):
            sv = xp[:, rows, v:v + W + 2 * HALF]           # [C, 100, 1290]
            win = sv[:, :, cols_idx]                       # [C, 100, 128, 20]
            xs[v] = win.transpose(2, 0, 1, 3).reshape(NP, SLABF).astype(BF16)
        in_maps.append({"k": kt_all[ci], "xs": xs, "ident": ident})
    return in_maps


def _assemble_y(results):
    """results[ci]["y"] is [128, 2700] bf16; reassemble to [1, C, H, W] f32."""
    y = np.empty((C, H, W), dtype=np.float32)
    for ci in range(NCORES):
        blk = np.asarray(results[ci]["y"], dtype=np.float32)
        blk = blk.reshape(NP, C, HS, CPP)                  # [p, c, r, w]
        y[:, HS * ci:HS * (ci + 1), :] = (
            blk.transpose(1, 2, 0, 3).reshape(C, HS, W))
    return y[None]


def kernel(x, k, padding, padding_value):
    in_maps = _prep_inputs(x, k, padding, padding_value)
    nc = get_nc()
    res = run_bass_kernel_spmd(nc, in_maps, core_ids=list(range(NCORES)))
    return _assemble_y(res.results).astype(np.float32)


# revision 9
# speedup vs baseline: 1.4007x; 1.4007x over previous
"""Trainium2 Bass kernel for per-pixel kernel application (KPN-style ApplyKernel).

y[c,h,w] = sum_{ii,jj} xpad[c, h+ii, w+jj] * k[ii*11+jj, h, w]

Strategy (8 NeuronCores, data-parallel over H strips of 90 rows):
  - Partition p owns a 10-column block of W (128 partitions x 10 = 1280), with
    the +-5 column halo stored in the free dim, so both row and column shifts
    of a tap are plain access-pattern offsets (DVE lanes are partition-locked,
    so shifts must live in the free dim). All 128 lanes are used.
  - Host: pad x and build bf16 slabs [128, 3ch x 100rows x 20cols] in two
    column-alignment variants so every tap's VectorE read stays 4-byte
    aligned, keeping tensor_tensor in its 2x bf16 mode. k is re-laid-out
    host-side to bf16 [128, 121, 900] (partition-block-major, even-column
    taps first) halving HBM traffic vs f32.
  - Taps are processed in 22 runs: for each row-shift ii, the 6 even-jj (or
    5 odd-jj) taps form one run. Per run and channel, ONE VectorE
    tensor_tensor with a 3-free-dim AP [p, tap(stride 2), row, col] computes
    all taps' products at once (bf16 2x mode, ~80ns instruction overhead
    amortized 6x). The final run falls back to per-tap ops so the PE tail
    stays short.
  - Products accumulate via 6 TensorE identity-matmuls per tap (K=M=128)
    into 6 PSUM banks (3 channels x 512/388-col chunks).
  - k runs are double-buffered on the SP HWDGE ring; slabs/ident fill via
    the ACT ring in parallel.  Output y is written bf16 (host casts to f32).
"""

import sys

if "/opt/trn_rl_repo" not in sys.path:
    sys.path.insert(0, "/opt/trn_rl_repo")

import numpy as np
import ml_dtypes

import concourse.mybir as mybir
from concourse import bacc
from concourse.tile import TileContext
from concourse.ap import AP
from concourse.bass_utils import run_bass_kernel_spmd

KS = 11
HALF = 5
H, W, C = 720, 1280, 3
NCORES = 8
HS = H // NCORES            # 90 rows per core
NP = 128                    # partitions (one 10-col block each)
CPP = W // NP               # 10 output cols per partition
ROWS_ST = HS + 2 * HALF     # 100 rows stored per partition
COLS_ST = CPP + 2 * HALF    # 20 cols stored per partition
SLABF = C * ROWS_ST * COLS_ST   # 6000 bf16 per partition per variant
NTAPS = KS * KS             # 121
FD = HS * CPP               # 900 elements per channel per tap
PFD = C * FD                # 2700 product elements per tap
N0, N1 = 512, FD - 512      # matmul chunk widths per channel (512, 388)

# tap order: even-jj taps first (only need slab variant 0), then odd-jj;
# within each half, ii-major so each run of same-ii taps is contiguous
TAP_PERM = ([t for t in range(NTAPS) if (t % KS) % 2 == 0]
            + [t for t in range(NTAPS) if (t % KS) % 2 == 1])
# runs of (start, ntaps): 11 x 6 even-jj, then 11 x 5 odd-jj
RUNS = ([(6 * i, 6) for i in range(11)]
        + [(66 + 5 * i, 5) for i in range(11)])

BF16 = ml_dtypes.bfloat16

_CACHE = {}


def _build_nc(taps=NTAPS):
    assert taps == NTAPS
    nc = bacc.Bacc("TRN2", target_bir_lowering=False, debug=False)
    k_d = nc.dram_tensor("k", [NP, NTAPS, FD], mybir.dt.bfloat16, kind="ExternalInput")
    xs_d = nc.dram_tensor("xs", [2, NP, SLABF], mybir.dt.bfloat16, kind="ExternalInput")
    id_d = nc.dram_tensor("ident", [NP, NP], mybir.dt.bfloat16, kind="ExternalInput")
    y_d = nc.dram_tensor("y", [NP, PFD], mybir.dt.bfloat16, kind="ExternalOutput")

    with TileContext(nc) as tc:
        with tc.tile_pool(name="const", bufs=1) as const_pool, \
             tc.tile_pool(name="kbf", bufs=3) as kb_pool, \
             tc.tile_pool(name="prod", bufs=2) as prod_pool, \
             tc.tile_pool(name="out", bufs=1) as out_pool, \
             tc.tile_pool(name="psum", bufs=1, space="PSUM") as psum_pool:

            slab0 = const_pool.tile([NP, SLABF], mybir.dt.bfloat16)
            slab1 = const_pool.tile([NP, SLABF], mybir.dt.bfloat16)
            ident = const_pool.tile([NP, NP], mybir.dt.bfloat16)

            kb_tiles = {}

            def dma_krun(r):
                gi0, nt = RUNS[r]
                kb = kb_pool.tile([NP, nt * FD], mybir.dt.bfloat16, name="kb")
                nc.sync.dma_start(
                    kb[:].rearrange("p (t f) -> p t f", t=nt),
                    k_d.ap()[:, gi0:gi0 + nt, :])
                kb_tiles[r] = kb

            # Fill: first k run on the SP ring; ident + slab halves on the
            # ACT ring so both rings stream in parallel.
            HL = SLABF // 2
            nc.scalar.dma_start(ident[:], id_d.ap())
            dma_krun(0)
            nc.scalar.dma_start(slab0[:, :HL], xs_d.ap()[0, :, :HL])
            nc.scalar.dma_start(slab0[:, HL:], xs_d.ap()[0, :, HL:])
            dma_krun(1)
            nc.scalar.dma_start(slab1[:, :HL], xs_d.ap()[1, :, :HL])
            nc.scalar.dma_start(slab1[:, HL:], xs_d.ap()[1, :, HL:])

            slab_views = [
                s[:].rearrange("p (c r w) -> p c r w", c=C, r=ROWS_ST, w=COLS_ST)
                for s in (slab0, slab1)]

            accs = []
            for c in range(C):
                a0 = psum_pool.tile([NP, N0], mybir.dt.float32, name=f"acc{c}0")
                a1 = psum_pool.tile([NP, N1], mybir.dt.float32, name=f"acc{c}1")
                accs.append((a0, a1))

            nrun = len(RUNS)
            for r in range(nrun):
                gi0, nt = RUNS[r]
                if r + 1 < nrun and r >= 1:
                    dma_krun(r + 1)
                kb = kb_tiles[r]
                t0 = TAP_PERM[gi0]
                ii = t0 // KS
                v = (t0 % KS) & 1
                sv = slab_views[v]

                prod = prod_pool.tile([NP, nt * PFD], mybir.dt.bfloat16,
                                      name="prod")
                prod_t = prod[:].rearrange("p (t f) -> p t f", t=nt)

                last_run = (r == nrun - 1)
                if not last_run:
                    # one 3-free-dim TT per channel covering all nt taps:
                    # x: [p, t(stride 2 cols), r, w]; k: [p, t(stride FD), r, w]
                    for c in range(C):
                        base = sv[:, c, ii:ii + HS, 0:CPP]
                        xs_op = AP(base.tensor, base.offset,
                                   [list(base.ap[0])] + [[2, nt]]
                                   + [list(x) for x in base.ap[1:]])
                        kap = kb[:]
                        k_op = AP(kap.tensor, kap.offset,
                                  [list(kap.ap[0])]
                                  + [[FD, nt], [CPP, HS], [1, CPP]])
                        po = prod[:].rearrange(
                            "p (t c r w) -> p t c r w", t=nt, c=C, r=HS, w=CPP
                        )[:, :, c]
                        nc.vector.tensor_tensor(po, xs_op, k_op,
                                                mybir.AluOpType.mult)
                else:
                    # per-tap ops so the final PE tail is one tap deep
                    for dt_ in range(nt):
                        t = TAP_PERM[gi0 + dt_]
                        jj2 = (t % KS) - v
                        xs_op = sv[:, :, ii:ii + HS, jj2:jj2 + CPP]
                        k_op = (kb[:, dt_ * FD:(dt_ + 1) * FD]
                                .rearrange("p (r w) -> p r w", r=HS)
                                .unsqueeze(1).broadcast_to([NP, C, HS, CPP]))
                        po = prod_t[:, dt_].rearrange(
                            "p (c r w) -> p c r w", c=C, r=HS, w=CPP)
                        nc.vector.tensor_tensor(po, xs_op, k_op,
                                                mybir.AluOpType.mult)

                for dt_ in range(nt):
                    gi = gi0 + dt_
                    first = (gi == 0)
                    last = (gi == taps - 1)
                    pt = prod_t[:, dt_]
                    for c in range(C):
                        nc.tensor.matmul(accs[c][0][:], ident[:],
                                         pt[:, c * FD:c * FD + N0],
                                         start=first, stop=last)
                        nc.tensor.matmul(accs[c][1][:], ident[:],
                                         pt[:, c * FD + N0:(c + 1) * FD],
                                         start=first, stop=last)

            yst = out_pool.tile([NP, PFD], mybir.dt.bfloat16)
            for c in range(C):
                nc.scalar.copy(yst[:, c * FD:c * FD + N0], accs[c][0][:])
                nc.vector.tensor_copy(yst[:, c * FD + N0:(c + 1) * FD],
                                      accs[c][1][:])
                nc.sync.dma_start(y_d.ap()[:, c * FD:(c + 1) * FD],
                                  yst[:, c * FD:(c + 1) * FD])

    nc.compile()
    return nc


def get_nc(taps=NTAPS):
    if taps not in _CACHE:
        _CACHE[taps] = _build_nc(taps)
    return _CACHE[taps]


def _prep_inputs(x, k, padding, padding_value):
    """Host-side prep: pad x, build bf16 slabs + per-core shards."""
    x = np.asarray(x, dtype=np.float32)
    k = np.asarray(k, dtype=np.float32)
    pad = bool(int(np.asarray(padding)))
    pv = float(np.asarray(padding_value))

    if pad:
        assert x.shape == (1, C, H, W), x.shape
        xp = np.full((C, H + 2 * HALF, W + 2 * HALF + 1), 0.0, dtype=np.float32)
        xp[:, :, :W + 2 * HALF] = pv
        xp[:, HALF:HALF + H, HALF:HALF + W] = x[0]
    else:
        assert x.shape == (1, C, H + 2 * HALF, W + 2 * HALF), x.shape
        xp = np.zeros((C, H + 2 * HALF, W + 2 * HALF + 1), dtype=np.float32)
        xp[:, :, :W + 2 * HALF] = x[0]

    assert k.shape == (1, NTAPS, H, W), k.shape
    # partition-block-major, tap-permuted k: [core, p, t, (r w)], bf16
    kt_all = np.ascontiguousarray(
        k[0][TAP_PERM].astype(BF16).reshape(NTAPS, NCORES, HS, NP, CPP)
        .transpose(1, 3, 0, 2, 4)).reshape(NCORES, NP, NTAPS, FD)

    cols_idx = CPP * np.arange(NP)[:, None] + np.arange(COLS_ST)[None, :]
    ident = np.eye(NP, dtype=BF16)
    in_maps = []
    for ci in range(NCORES):
        rows = slice(HS * ci, HS * ci + ROWS_ST)
        xs = np.empty((2, NP, SLABF), dtype=BF16)
        for v in (0, 1):
            sv = xp[:, rows, v:v + W + 2 * HALF]           # [C, 100, 1290]
            win = sv[:, :, cols_idx]                       # [C, 100, 128, 20]
            xs[v] = win.transpose(2, 0, 1, 3).reshape(NP, SLABF).astype(BF16)
        in_maps.append({"k": kt_all[ci], "xs": xs, "ident": ident})
    return in_maps


def _assemble_y(results):
    """results[ci]["y"] is [128, 2700] bf16; reassemble to [1, C, H, W] f32."""
    y = np.empty((C, H, W), dtype=np.float32)
    for ci in range(NCORES):
        blk = np.asarray(results[ci]["y"], dtype=np.float32)
        blk = blk.reshape(NP, C, HS, CPP)                  # [p, c, r, w]
        y[:, HS * ci:HS * (ci + 1), :] = (
            blk.transpose(1, 2, 0, 3).reshape(C, HS, W))
    return y[None]


def kernel(x, k, padding, padding_value):
    in_maps = _prep_inputs(x, k, padding, padding_value)
    nc = get_nc()
    res = run_bass_kernel_spmd(nc, in_maps, core_ids=list(range(NCORES)))
    return _assemble_y(res.results).astype(np.float32)


# revision 14
# speedup vs baseline: 1.4173x; 1.0119x over previous
"""Trainium2 Bass kernel for per-pixel kernel application (KPN-style ApplyKernel).

y[c,h,w] = sum_{ii,jj} xpad[c, h+ii, w+jj] * k[ii*11+jj, h, w]

Strategy (8 NeuronCores, data-parallel over H strips of 90 rows):
  - Partition p owns a 10-column block of W (128 partitions x 10 = 1280), with
    the +-5 column halo stored in the free dim, so both row and column shifts
    of a tap are plain access-pattern offsets (DVE lanes are partition-locked,
    so shifts must live in the free dim). All 128 lanes are used.
  - Host: pad x and build bf16 slabs [128, 3ch x 100rows x 20cols] in two
    column-alignment variants so every tap's VectorE read stays 4-byte
    aligned, keeping tensor_tensor in its 2x bf16 mode. k is re-laid-out
    host-side to bf16 [128, 121, 900] (partition-block-major, even-column
    taps first) halving HBM traffic vs f32.
  - Taps are processed in 22 runs: for each row-shift ii, the 6 even-jj (or
    5 odd-jj) taps form one run. Per run and channel, ONE VectorE
    tensor_tensor with a 3-free-dim AP [p, tap(stride 2), row, col] computes
    all taps' products at once (bf16 2x mode, ~80ns instruction overhead
    amortized 6x). The final run falls back to per-tap ops so the PE tail
    stays short.
  - Products accumulate via 6 TensorE identity-matmuls per tap (K=M=128)
    into 6 PSUM banks (3 channels x 512/388-col chunks).
  - k runs are double-buffered on the SP HWDGE ring; slabs/ident fill via
    the ACT ring in parallel.  Output y is written bf16 (host casts to f32).
"""

import sys

if "/opt/trn_rl_repo" not in sys.path:
    sys.path.insert(0, "/opt/trn_rl_repo")

import numpy as np
import ml_dtypes

import concourse.mybir as mybir
from concourse import bacc
from concourse.tile import TileContext
from concourse.ap import AP
from concourse.bass_utils import run_bass_kernel_spmd

KS = 11
HALF = 5
H, W, C = 720, 1280, 3
NCORES = 8
HS = H // NCORES            # 90 rows per core
NP = 128                    # partitions (one 10-col block each)
CPP = W // NP               # 10 output cols per partition
ROWS_ST = HS + 2 * HALF     # 100 rows stored per partition
COLS_ST = CPP + 2 * HALF    # 20 cols stored per partition
SLABF = C * ROWS_ST * COLS_ST   # 6000 bf16 per partition per variant
NTAPS = KS * KS             # 121
FD = HS * CPP               # 900 elements per channel per tap
PFD = C * FD                # 2700 product elements per tap
N0, N1 = 512, FD - 512      # matmul chunk widths per channel (512, 388)

# tap order: even-jj taps first (only need slab variant 0), then odd-jj;
# within each half, ii-major so each run of same-ii taps is contiguous
TAP_PERM = ([t for t in range(NTAPS) if (t % KS) % 2 == 0]
            + [t for t in range(NTAPS) if (t % KS) % 2 == 1])
# runs of (start, ntaps): first run split 3+3 so the fill-phase k DMA is
# small, then 10 x 6 even-jj, then 11 x 5 odd-jj
RUNS = ([(0, 3), (3, 3)] + [(6 * i, 6) for i in range(1, 11)]
        + [(66 + 5 * i, 5) for i in range(11)])
CF = ROWS_ST * COLS_ST      # 2000 elements per channel slab chunk

BF16 = ml_dtypes.bfloat16

_CACHE = {}


def _build_nc(taps=NTAPS):
    assert taps == NTAPS
    nc = bacc.Bacc("TRN2", target_bir_lowering=False, debug=False)
    k_d = nc.dram_tensor("k", [NP, NTAPS, FD], mybir.dt.bfloat16, kind="ExternalInput")
    xs_d = nc.dram_tensor("xs", [2, NP, SLABF], mybir.dt.bfloat16, kind="ExternalInput")
    id_d = nc.dram_tensor("ident", [NP, NP], mybir.dt.bfloat16, kind="ExternalInput")
    y_d = nc.dram_tensor("y", [NP, PFD], mybir.dt.bfloat16, kind="ExternalOutput")

    with TileContext(nc) as tc:
        with tc.tile_pool(name="const", bufs=1) as const_pool, \
             tc.tile_pool(name="kbf", bufs=3) as kb_pool, \
             tc.tile_pool(name="prod", bufs=2) as prod_pool, \
             tc.tile_pool(name="out", bufs=1) as out_pool, \
             tc.tile_pool(name="psum", bufs=1, space="PSUM") as psum_pool:

            # per-(variant, channel) slab tiles so the first multiply only
            # waits on its own channel's DMA
            slabs = [[const_pool.tile([NP, CF], mybir.dt.bfloat16,
                                      name=f"slab{v}c{c}")
                      for c in range(C)] for v in range(2)]
            ident = const_pool.tile([NP, NP], mybir.dt.bfloat16)

            kb_tiles = {}

            def dma_krun(r, eng):
                gi0, nt = RUNS[r]
                kb = kb_pool.tile([NP, nt * FD], mybir.dt.bfloat16, name="kb")
                eng.dma_start(
                    kb[:].rearrange("p (t f) -> p t f", t=nt),
                    k_d.ap()[:, gi0:gi0 + nt, :])
                kb_tiles[r] = kb

            # Fill: the critical path (ident, slab0-c0, k run 0) goes first
            # on the SP ring (it starts ~2.5us earlier than ACT); slab1 on
            # the ACT ring in parallel.
            xsv = xs_d.ap().rearrange("v p (c f) -> v p c f", c=C)
            nc.sync.dma_start(ident[:], id_d.ap())
            nc.sync.dma_start(slabs[0][0][:], xsv[0, :, 0])
            dma_krun(0, nc.sync)
            nc.sync.dma_start(slabs[0][1][:], xsv[0, :, 1])
            dma_krun(1, nc.sync)
            nc.sync.dma_start(slabs[0][2][:], xsv[0, :, 2])
            for c in range(C):
                nc.scalar.dma_start(slabs[1][c][:], xsv[1, :, c])

            slab_views = [
                [slabs[v][c][:].rearrange("p (r w) -> p r w", r=ROWS_ST)
                 for c in range(C)] for v in range(2)]

            accs = []
            for c in range(C):
                a0 = psum_pool.tile([NP, N0], mybir.dt.float32, name=f"acc{c}0")
                a1 = psum_pool.tile([NP, N1], mybir.dt.float32, name=f"acc{c}1")
                accs.append((a0, a1))

            nrun = len(RUNS)
            for r in range(nrun):
                gi0, nt = RUNS[r]
                if r + 1 < nrun and r >= 1:
                    dma_krun(r + 1, nc.sync)
                kb = kb_tiles[r]
                t0 = TAP_PERM[gi0]
                ii = t0 // KS
                v = (t0 % KS) & 1

                prod = prod_pool.tile([NP, nt * PFD], mybir.dt.bfloat16,
                                      name="prod")
                prod_t = prod[:].rearrange("p (t f) -> p t f", t=nt)

                last_run = (r == nrun - 1)
                if not last_run:
                    # one 3-free-dim TT per channel covering all nt taps:
                    # x: [p, t(stride 2 cols), r, w]; k: [p, t(stride FD), r, w]
                    jb = (TAP_PERM[gi0] % KS) - v   # first tap's jj2
                    for c in range(C):
                        base = slab_views[v][c][:, ii:ii + HS, jb:jb + CPP]
                        xs_op = AP(base.tensor, base.offset,
                                   [list(base.ap[0])] + [[2, nt]]
                                   + [list(x) for x in base.ap[1:]])
                        kap = kb[:]
                        k_op = AP(kap.tensor, kap.offset,
                                  [list(kap.ap[0])]
                                  + [[FD, nt], [CPP, HS], [1, CPP]])
                        po = prod[:].rearrange(
                            "p (t c r w) -> p t c r w", t=nt, c=C, r=HS, w=CPP
                        )[:, :, c]
                        nc.vector.tensor_tensor(po, xs_op, k_op,
                                                mybir.AluOpType.mult)
                else:
                    # per-tap, per-channel ops so the final PE tail is short
                    for dt_ in range(nt):
                        t = TAP_PERM[gi0 + dt_]
                        jj2 = (t % KS) - v
                        for c in range(C):
                            xs_op = slab_views[v][c][:, ii:ii + HS,
                                                     jj2:jj2 + CPP]
                            k_op = (kb[:, dt_ * FD:(dt_ + 1) * FD]
                                    .rearrange("p (r w) -> p r w", r=HS))
                            po = prod_t[:, dt_].rearrange(
                                "p (c r w) -> p c r w", c=C, r=HS, w=CPP
                            )[:, c]
                            nc.vector.tensor_tensor(po, xs_op, k_op,
                                                    mybir.AluOpType.mult)

                for dt_ in range(nt):
                    gi = gi0 + dt_
                    first = (gi == 0)
                    last = (gi == taps - 1)
                    pt = prod_t[:, dt_]
                    for c in range(C):
                        nc.tensor.matmul(accs[c][0][:], ident[:],
                                         pt[:, c * FD:c * FD + N0],
                                         start=first, stop=last)
                        nc.tensor.matmul(accs[c][1][:], ident[:],
                                         pt[:, c * FD + N0:(c + 1) * FD],
                                         start=first, stop=last)

            yst = out_pool.tile([NP, PFD], mybir.dt.bfloat16)
            for c in range(C):
                nc.scalar.copy(yst[:, c * FD:c * FD + N0], accs[c][0][:])
                nc.vector.tensor_copy(yst[:, c * FD + N0:(c + 1) * FD],
                                      accs[c][1][:])
                nc.sync.dma_start(y_d.ap()[:, c * FD:(c + 1) * FD],
                                  yst[:, c * FD:(c + 1) * FD])

    nc.compile()
    return nc


def get_nc(taps=NTAPS):
    if taps not in _CACHE:
        _CACHE[taps] = _build_nc(taps)
    return _CACHE[taps]


def _prep_inputs(x, k, padding, padding_value):
    """Host-side prep: pad x, build bf16 slabs + per-core shards."""
    x = np.asarray(x, dtype=np.float32)
    k = np.asarray(k, dtype=np.float32)
    pad = bool(int(np.asarray(padding)))
    pv = float(np.asarray(padding_value))

    if pad:
        assert x.shape == (1, C, H, W), x.shape
        xp = np.full((C, H + 2 * HALF, W + 2 * HALF + 1), 0.0, dtype=np.float32)
        xp[:, :, :W + 2 * HALF] = pv
        xp[:, HALF:HALF + H, HALF:HALF + W] = x[0]
    else:
        assert x.shape == (1, C, H + 2 * HALF, W + 2 * HALF), x.shape
        xp = np.zeros((C, H + 2 * HALF, W + 2 * HALF + 1), dtype=np.float32)
        xp[:, :, :W + 2 * HALF] = x[0]

    assert k.shape == (1, NTAPS, H, W), k.shape
    # partition-block-major, tap-permuted k: [core, p, t, (r w)], bf16
    kt_all = np.ascontiguousarray(
        k[0][TAP_PERM].astype(BF16).reshape(NTAPS, NCORES, HS, NP, CPP)
        .transpose(1, 3, 0, 2, 4)).reshape(NCORES, NP, NTAPS, FD)

    cols_idx = CPP * np.arange(NP)[:, None] + np.arange(COLS_ST)[None, :]
    ident = np.eye(NP, dtype=BF16)
    in_maps = []
    for ci in range(NCORES):
        rows = slice(HS * ci, HS * ci + ROWS_ST)
        xs = np.empty((2, NP, SLABF), dtype=BF16)
        for v in (0, 1):
            sv = xp[:, rows, v:v + W + 2 * HALF]           # [C, 100, 1290]
            win = sv[:, :, cols_idx]                       # [C, 100, 128, 20]
            xs[v] = win.transpose(2, 0, 1, 3).reshape(NP, SLABF).astype(BF16)
        in_maps.append({"k": kt_all[ci], "xs": xs, "ident": ident})
    return in_maps


def _assemble_y(results):
    """results[ci]["y"] is [128, 2700] bf16; reassemble to [1, C, H, W] f32."""
    y = np.empty((C, H, W), dtype=np.float32)
    for ci in range(NCORES):
        blk = np.asarray(results[ci]["y"], dtype=np.float32)
        blk = blk.reshape(NP, C, HS, CPP)                  # [p, c, r, w]
        y[:, HS * ci:HS * (ci + 1), :] = (
            blk.transpose(1, 2, 0, 3).reshape(C, HS, W))
    return y[None]


def kernel(x, k, padding, padding_value):
    in_maps = _prep_inputs(x, k, padding, padding_value)
    nc = get_nc()
    res = run_bass_kernel_spmd(nc, in_maps, core_ids=list(range(NCORES)))
    return _assemble_y(res.results).astype(np.float32)


# revision 16
# speedup vs baseline: 1.4181x; 1.0005x over previous
"""Trainium2 Bass kernel for per-pixel kernel application (KPN-style ApplyKernel).

y[c,h,w] = sum_{ii,jj} xpad[c, h+ii, w+jj] * k[ii*11+jj, h, w]

Strategy (8 NeuronCores, data-parallel over H strips of 90 rows):
  - Partition p owns a 10-column block of W (128 partitions x 10 = 1280), with
    the +-5 column halo stored in the free dim, so both row and column shifts
    of a tap are plain access-pattern offsets (DVE lanes are partition-locked,
    so shifts must live in the free dim). All 128 lanes are used.
  - Host: pad x and build bf16 slabs [128, 3ch x 100rows x 20cols] in two
    column-alignment variants so every tap's VectorE read stays 4-byte
    aligned, keeping tensor_tensor in its 2x bf16 mode. k is re-laid-out
    host-side to bf16 [128, 121, 900] (partition-block-major, even-column
    taps first) halving HBM traffic vs f32.
  - Taps are processed in 22 runs: for each row-shift ii, the 6 even-jj (or
    5 odd-jj) taps form one run. Per run and channel, ONE VectorE
    tensor_tensor with a 3-free-dim AP [p, tap(stride 2), row, col] computes
    all taps' products at once (bf16 2x mode, ~80ns instruction overhead
    amortized 6x). The final run falls back to per-tap ops so the PE tail
    stays short.
  - Products accumulate via 6 TensorE identity-matmuls per tap (K=M=128)
    into 6 PSUM banks (3 channels x 512/388-col chunks).
  - k runs are double-buffered on the SP HWDGE ring; slabs/ident fill via
    the ACT ring in parallel.  Output y is written bf16 (host casts to f32).
"""

import sys

if "/opt/trn_rl_repo" not in sys.path:
    sys.path.insert(0, "/opt/trn_rl_repo")

import numpy as np
import ml_dtypes

import concourse.mybir as mybir
from concourse import bacc
from concourse.tile import TileContext
from concourse.ap import AP
from concourse.bass_utils import run_bass_kernel_spmd

KS = 11
HALF = 5
H, W, C = 720, 1280, 3
NCORES = 8
HS = H // NCORES            # 90 rows per core
NP = 128                    # partitions (one 10-col block each)
CPP = W // NP               # 10 output cols per partition
ROWS_ST = HS + 2 * HALF     # 100 rows stored per partition
COLS_ST = CPP + 2 * HALF    # 20 cols stored per partition
SLABF = C * ROWS_ST * COLS_ST   # 6000 bf16 per partition per variant
NTAPS = KS * KS             # 121
FD = HS * CPP               # 900 elements per channel per tap
PFD = C * FD                # 2700 product elements per tap
N0, N1 = 512, FD - 512      # matmul chunk widths per channel (512, 388)

# tap order: even-jj taps first (only need slab variant 0), then odd-jj;
# within each half, ii-major so each run of same-ii taps is contiguous
TAP_PERM = ([t for t in range(NTAPS) if (t % KS) % 2 == 0]
            + [t for t in range(NTAPS) if (t % KS) % 2 == 1])
# runs of (start, ntaps): first run split 3+3 so the fill-phase k DMA is
# small, then 10 x 6 even-jj, then 11 x 5 odd-jj
RUNS = ([(0, 2), (2, 4)] + [(6 * i, 6) for i in range(1, 11)]
        + [(66 + 5 * i, 5) for i in range(11)])
CF = ROWS_ST * COLS_ST      # 2000 elements per channel slab chunk

BF16 = ml_dtypes.bfloat16

_CACHE = {}


def _build_nc(taps=NTAPS):
    assert taps == NTAPS
    nc = bacc.Bacc("TRN2", target_bir_lowering=False, debug=False)
    k_d = nc.dram_tensor("k", [NP, NTAPS, FD], mybir.dt.bfloat16, kind="ExternalInput")
    xs_d = nc.dram_tensor("xs", [2, NP, SLABF], mybir.dt.bfloat16, kind="ExternalInput")
    id_d = nc.dram_tensor("ident", [NP, NP], mybir.dt.bfloat16, kind="ExternalInput")
    y_d = nc.dram_tensor("y", [NP, PFD], mybir.dt.bfloat16, kind="ExternalOutput")

    with TileContext(nc) as tc:
        with tc.tile_pool(name="const", bufs=1) as const_pool, \
             tc.tile_pool(name="kbf", bufs=3) as kb_pool, \
             tc.tile_pool(name="prod", bufs=2) as prod_pool, \
             tc.tile_pool(name="out", bufs=1) as out_pool, \
             tc.tile_pool(name="psum", bufs=1, space="PSUM") as psum_pool:

            # per-(variant, channel) slab tiles so the first multiply only
            # waits on its own channel's DMA
            slabs = [[const_pool.tile([NP, CF], mybir.dt.bfloat16,
                                      name=f"slab{v}c{c}")
                      for c in range(C)] for v in range(2)]
            ident = const_pool.tile([NP, NP], mybir.dt.bfloat16)

            kb_tiles = {}

            def dma_krun(r, eng):
                gi0, nt = RUNS[r]
                kb = kb_pool.tile([NP, nt * FD], mybir.dt.bfloat16, name="kb")
                eng.dma_start(
                    kb[:].rearrange("p (t f) -> p t f", t=nt),
                    k_d.ap()[:, gi0:gi0 + nt, :])
                kb_tiles[r] = kb

            # Fill: the critical path (ident, slab0-c0, k run 0) goes first
            # on the SP ring (it starts ~2.5us earlier than ACT); slab1 on
            # the ACT ring in parallel.
            xsv = xs_d.ap().rearrange("v p (c f) -> v p c f", c=C)
            dma_krun(0, nc.sync)
            nc.sync.dma_start(slabs[0][0][:], xsv[0, :, 0])
            dma_krun(1, nc.sync)
            nc.sync.dma_start(slabs[0][1][:], xsv[0, :, 1])
            nc.sync.dma_start(slabs[0][2][:], xsv[0, :, 2])
            nc.scalar.dma_start(ident[:], id_d.ap())
            for c in range(C):
                nc.scalar.dma_start(slabs[1][c][:], xsv[1, :, c])

            slab_views = [
                [slabs[v][c][:].rearrange("p (r w) -> p r w", r=ROWS_ST)
                 for c in range(C)] for v in range(2)]

            accs = []
            for c in range(C):
                a0 = psum_pool.tile([NP, N0], mybir.dt.float32, name=f"acc{c}0")
                a1 = psum_pool.tile([NP, N1], mybir.dt.float32, name=f"acc{c}1")
                accs.append((a0, a1))

            nrun = len(RUNS)
            for r in range(nrun):
                gi0, nt = RUNS[r]
                if r + 1 < nrun and r >= 1:
                    dma_krun(r + 1, nc.sync)
                kb = kb_tiles[r]
                t0 = TAP_PERM[gi0]
                ii = t0 // KS
                v = (t0 % KS) & 1

                prod = prod_pool.tile([NP, nt * PFD], mybir.dt.bfloat16,
                                      name="prod")
                prod_t = prod[:].rearrange("p (t f) -> p t f", t=nt)

                last_run = (r == nrun - 1)
                if not last_run:
                    # one 3-free-dim TT per channel covering all nt taps:
                    # x: [p, t(stride 2 cols), r, w]; k: [p, t(stride FD), r, w]
                    jb = (TAP_PERM[gi0] % KS) - v   # first tap's jj2
                    for c in range(C):
                        base = slab_views[v][c][:, ii:ii + HS, jb:jb + CPP]
                        xs_op = AP(base.tensor, base.offset,
                                   [list(base.ap[0])] + [[2, nt]]
                                   + [list(x) for x in base.ap[1:]])
                        kap = kb[:]
                        k_op = AP(kap.tensor, kap.offset,
                                  [list(kap.ap[0])]
                                  + [[FD, nt], [CPP, HS], [1, CPP]])
                        po = prod[:].rearrange(
                            "p (t c r w) -> p t c r w", t=nt, c=C, r=HS, w=CPP
                        )[:, :, c]
                        nc.vector.tensor_tensor(po, xs_op, k_op,
                                                mybir.AluOpType.mult)
                else:
                    # per-tap, per-channel ops so the final PE tail is short
                    for dt_ in range(nt):
                        t = TAP_PERM[gi0 + dt_]
                        jj2 = (t % KS) - v
                        for c in range(C):
                            xs_op = slab_views[v][c][:, ii:ii + HS,
                                                     jj2:jj2 + CPP]
                            k_op = (kb[:, dt_ * FD:(dt_ + 1) * FD]
                                    .rearrange("p (r w) -> p r w", r=HS))
                            po = prod_t[:, dt_].rearrange(
                                "p (c r w) -> p c r w", c=C, r=HS, w=CPP
                            )[:, c]
                            nc.vector.tensor_tensor(po, xs_op, k_op,
                                                    mybir.AluOpType.mult)

                for dt_ in range(nt):
                    gi = gi0 + dt_
                    first = (gi == 0)
                    last = (gi == taps - 1)
                    pt = prod_t[:, dt_]
                    for c in range(C):
                        nc.tensor.matmul(accs[c][0][:], ident[:],
                                         pt[:, c * FD:c * FD + N0],
                                         start=first, stop=last)
                        nc.tensor.matmul(accs[c][1][:], ident[:],
                                         pt[:, c * FD + N0:(c + 1) * FD],
                                         start=first, stop=last)

            yst = out_pool.tile([NP, PFD], mybir.dt.bfloat16)
            for c in range(C):
                nc.scalar.copy(yst[:, c * FD:c * FD + N0], accs[c][0][:])
                nc.vector.tensor_copy(yst[:, c * FD + N0:(c + 1) * FD],
                                      accs[c][1][:])
                nc.sync.dma_start(y_d.ap()[:, c * FD:(c + 1) * FD],
                                  yst[:, c * FD:(c + 1) * FD])

    nc.compile()
    return nc


def get_nc(taps=NTAPS):
    if taps not in _CACHE:
        _CACHE[taps] = _build_nc(taps)
    return _CACHE[taps]


def _prep_inputs(x, k, padding, padding_value):
    """Host-side prep: pad x, build bf16 slabs + per-core shards."""
    x = np.asarray(x, dtype=np.float32)
    k = np.asarray(k, dtype=np.float32)
    pad = bool(int(np.asarray(padding)))
    pv = float(np.asarray(padding_value))

    if pad:
        assert x.shape == (1, C, H, W), x.shape
        xp = np.full((C, H + 2 * HALF, W + 2 * HALF + 1), 0.0, dtype=np.float32)
        xp[:, :, :W + 2 * HALF] = pv
        xp[:, HALF:HALF + H, HALF:HALF + W] = x[0]
    else:
        assert x.shape == (1, C, H + 2 * HALF, W + 2 * HALF), x.shape
        xp = np.zeros((C, H + 2 * HALF, W + 2 * HALF + 1), dtype=np.float32)
        xp[:, :, :W + 2 * HALF] = x[0]

    assert k.shape == (1, NTAPS, H, W), k.shape
    # partition-block-major, tap-permuted k: [core, p, t, (r w)], bf16
    kt_all = np.ascontiguousarray(
        k[0][TAP_PERM].astype(BF16).reshape(NTAPS, NCORES, HS, NP, CPP)
        .transpose(1, 3, 0, 2, 4)).reshape(NCORES, NP, NTAPS, FD)

    cols_idx = CPP * np.arange(NP)[:, None] + np.arange(COLS_ST)[None, :]
    ident = np.eye(NP, dtype=BF16)
    in_maps = []
    for ci in range(NCORES):
        rows = slice(HS * ci, HS * ci + ROWS_ST)
        xs = np.empty((2, NP, SLABF), dtype=BF16)
        for v in (0, 1):
            sv = xp[:, rows, v:v + W + 2 * HALF]           # [C, 100, 1290]
            win = sv[:, :, cols_idx]                       # [C, 100, 128, 20]
            xs[v] = win.transpose(2, 0, 1, 3).reshape(NP, SLABF).astype(BF16)
        in_maps.append({"k": kt_all[ci], "xs": xs, "ident": ident})
    return in_maps


def _assemble_y(results):
    """results[ci]["y"] is [128, 2700] bf16; reassemble to [1, C, H, W] f32."""
    y = np.empty((C, H, W), dtype=np.float32)
    for ci in range(NCORES):
        blk = np.asarray(results[ci]["y"], dtype=np.float32)
        blk = blk.reshape(NP, C, HS, CPP)                  # [p, c, r, w]
        y[:, HS * ci:HS * (ci + 1), :] = (
            blk.transpose(1, 2, 0, 3).reshape(C, HS, W))
    return y[None]


def kernel(x, k, padding, padding_value):
    in_maps = _prep_inputs(x, k, padding, padding_value)
    nc = get_nc()
    res = run_bass_kernel_spmd(nc, in_maps, core_ids=list(range(NCORES)))
    return _assemble_y(res.results).astype(np.float32)


# revision 20
# speedup vs baseline: 1.4264x; 1.0059x over previous
"""Trainium2 Bass kernel for per-pixel kernel application (KPN-style ApplyKernel).

y[c,h,w] = sum_{ii,jj} xpad[c, h+ii, w+jj] * k[ii*11+jj, h, w]

Strategy (8 NeuronCores, data-parallel over H strips of 90 rows):
  - Partition p owns a 10-column block of W (128 partitions x 10 = 1280), with
    the +-5 column halo stored in the free dim, so both row and column shifts
    of a tap are plain access-pattern offsets (DVE lanes are partition-locked,
    so shifts must live in the free dim). All 128 lanes are used.
  - Host: pad x and build bf16 slabs [128, 3ch x 100rows x 20cols] in two
    column-alignment variants so every tap's VectorE read stays 4-byte
    aligned, keeping tensor_tensor in its 2x bf16 mode. k is re-laid-out
    host-side to bf16 [128, 121, 900] (partition-block-major, even-column
    taps first) halving HBM traffic vs f32.
  - Taps are processed in 22 runs: for each row-shift ii, the 6 even-jj (or
    5 odd-jj) taps form one run. Per run and channel, ONE VectorE
    tensor_tensor with a 3-free-dim AP [p, tap(stride 2), row, col] computes
    all taps' products at once (bf16 2x mode, ~80ns instruction overhead
    amortized 6x). The final run falls back to per-tap ops so the PE tail
    stays short.
  - Products accumulate via 6 TensorE identity-matmuls per tap (K=M=128)
    into 6 PSUM banks (3 channels x 512/388-col chunks).
  - k runs are double-buffered on the SP HWDGE ring; slabs/ident fill via
    the ACT ring in parallel.  Output y is written bf16 (host casts to f32).
"""

import sys

if "/opt/trn_rl_repo" not in sys.path:
    sys.path.insert(0, "/opt/trn_rl_repo")

import numpy as np
import ml_dtypes

import concourse.mybir as mybir
from concourse import bacc
from concourse.tile import TileContext
from concourse.ap import AP
from concourse.bass_utils import run_bass_kernel_spmd

KS = 11
HALF = 5
H, W, C = 720, 1280, 3
NCORES = 8
HS = H // NCORES            # 90 rows per core
NP = 128                    # partitions (one 10-col block each)
CPP = W // NP               # 10 output cols per partition
ROWS_ST = HS + 2 * HALF     # 100 rows stored per partition
COLS_ST = CPP + 2 * HALF    # 20 cols stored per partition
SLABF = C * ROWS_ST * COLS_ST   # 6000 bf16 per partition per variant
NTAPS = KS * KS             # 121
FD = HS * CPP               # 900 elements per channel per tap
PFD = C * FD                # 2700 product elements per tap
N0, N1 = 512, FD - 512      # matmul chunk widths per channel (512, 388)

# tap order: even-jj taps first (only need slab variant 0), then odd-jj;
# within each half, ii-major so each run of same-ii taps is contiguous
TAP_PERM = ([t for t in range(NTAPS) if (t % KS) % 2 == 0]
            + [t for t in range(NTAPS) if (t % KS) % 2 == 1])
# runs of (start, ntaps): first run split 3+3 so the fill-phase k DMA is
# small, then 10 x 6 even-jj, then 11 x 5 odd-jj
RUNS = ([(0, 2), (2, 4)] + [(6 * i, 6) for i in range(1, 11)]
        + [(66 + 5 * i, 5) for i in range(11)])
CF = ROWS_ST * COLS_ST      # 2000 elements per channel slab chunk

BF16 = ml_dtypes.bfloat16

_CACHE = {}


def _build_nc(taps=NTAPS):
    assert taps == NTAPS
    nc = bacc.Bacc("TRN2", target_bir_lowering=False, debug=False)
    k_d = nc.dram_tensor("k", [NP, NTAPS, FD], mybir.dt.bfloat16, kind="ExternalInput")
    xs_d = nc.dram_tensor("xs", [2, NP, SLABF], mybir.dt.bfloat16, kind="ExternalInput")
    id_d = nc.dram_tensor("ident", [NP, NP], mybir.dt.bfloat16, kind="ExternalInput")
    y_d = nc.dram_tensor("y", [NP, PFD], mybir.dt.bfloat16, kind="ExternalOutput")

    with TileContext(nc) as tc:
        with tc.tile_pool(name="const", bufs=1) as const_pool, \
             tc.tile_pool(name="kbf", bufs=4) as kb_pool, \
             tc.tile_pool(name="prod", bufs=2) as prod_pool, \
             tc.tile_pool(name="out", bufs=1) as out_pool, \
             tc.tile_pool(name="psum", bufs=1, space="PSUM") as psum_pool:

            # per-(variant, channel) slab tiles so the first multiply only
            # waits on its own channel's DMA
            slabs = [[const_pool.tile([NP, CF], mybir.dt.bfloat16,
                                      name=f"slab{v}c{c}")
                      for c in range(C)] for v in range(2)]
            ident = const_pool.tile([NP, NP], mybir.dt.bfloat16)

            kb_tiles = {}

            def dma_krun(r, eng):
                gi0, nt = RUNS[r]
                kb = kb_pool.tile([NP, nt * FD], mybir.dt.bfloat16, name="kb")
                eng.dma_start(
                    kb[:].rearrange("p (t f) -> p t f", t=nt),
                    k_d.ap()[:, gi0:gi0 + nt, :])
                kb_tiles[r] = kb

            # Fill: the critical path (ident, slab0-c0, k run 0) goes first
            # on the SP ring (it starts ~2.5us earlier than ACT); slab1 on
            # the ACT ring in parallel.
            xsv = xs_d.ap().rearrange("v p (c f) -> v p c f", c=C)
            dma_krun(0, nc.sync)
            nc.sync.dma_start(slabs[0][0][:], xsv[0, :, 0])
            dma_krun(1, nc.sync)
            dma_krun(2, nc.sync)
            nc.scalar.dma_start(slabs[0][1][:], xsv[0, :, 1])
            nc.scalar.dma_start(slabs[0][2][:], xsv[0, :, 2])
            nc.scalar.dma_start(ident[:], id_d.ap())
            for c in range(C):
                nc.scalar.dma_start(slabs[1][c][:], xsv[1, :, c])

            slab_views = [
                [slabs[v][c][:].rearrange("p (r w) -> p r w", r=ROWS_ST)
                 for c in range(C)] for v in range(2)]

            accs = []
            for c in range(C):
                a0 = psum_pool.tile([NP, N0], mybir.dt.float32, name=f"acc{c}0")
                a1 = psum_pool.tile([NP, N1], mybir.dt.float32, name=f"acc{c}1")
                accs.append((a0, a1))

            nrun = len(RUNS)
            for r in range(nrun):
                gi0, nt = RUNS[r]
                if r + 2 < nrun and r >= 1:
                    dma_krun(r + 2, nc.sync)
                kb = kb_tiles[r]
                t0 = TAP_PERM[gi0]
                ii = t0 // KS
                v = (t0 % KS) & 1

                prod = prod_pool.tile([NP, nt * PFD], mybir.dt.bfloat16,
                                      name="prod")
                prod_t = prod[:].rearrange("p (t f) -> p t f", t=nt)

                last_run = (r == nrun - 1)
                if not last_run:
                    # one 3-free-dim TT per channel covering all nt taps:
                    # x: [p, t(stride 2 cols), r, w]; k: [p, t(stride FD), r, w]
                    jb = (TAP_PERM[gi0] % KS) - v   # first tap's jj2
                    for c in range(C):
                        base = slab_views[v][c][:, ii:ii + HS, jb:jb + CPP]
                        xs_op = AP(base.tensor, base.offset,
                                   [list(base.ap[0])] + [[2, nt]]
                                   + [list(x) for x in base.ap[1:]])
                        kap = kb[:]
                        k_op = AP(kap.tensor, kap.offset,
                                  [list(kap.ap[0])]
                                  + [[FD, nt], [CPP, HS], [1, CPP]])
                        po = prod[:].rearrange(
                            "p (t c r w) -> p t c r w", t=nt, c=C, r=HS, w=CPP
                        )[:, :, c]
                        nc.vector.tensor_tensor(po, xs_op, k_op,
                                                mybir.AluOpType.mult)
                else:
                    # per-tap, per-channel ops so the final PE tail is short
                    for dt_ in range(nt):
                        t = TAP_PERM[gi0 + dt_]
                        jj2 = (t % KS) - v
                        for c in range(C):
                            xs_op = slab_views[v][c][:, ii:ii + HS,
                                                     jj2:jj2 + CPP]
                            k_op = (kb[:, dt_ * FD:(dt_ + 1) * FD]
                                    .rearrange("p (r w) -> p r w", r=HS))
                            po = prod_t[:, dt_].rearrange(
                                "p (c r w) -> p c r w", c=C, r=HS, w=CPP
                            )[:, c]
                            nc.vector.tensor_tensor(po, xs_op, k_op,
                                                    mybir.AluOpType.mult)

                for dt_ in range(nt):
                    gi = gi0 + dt_
                    first = (gi == 0)
                    last = (gi == taps - 1)
                    pt = prod_t[:, dt_]
                    for c in range(C):
                        nc.tensor.matmul(accs[c][0][:], ident[:],
                                         pt[:, c * FD:c * FD + N0],
                                         start=first, stop=last)
                        nc.tensor.matmul(accs[c][1][:], ident[:],
                                         pt[:, c * FD + N0:(c + 1) * FD],
                                         start=first, stop=last)

            yst = out_pool.tile([NP, PFD], mybir.dt.bfloat16)
            for c in range(C):
                nc.scalar.copy(yst[:, c * FD:c * FD + N0], accs[c][0][:])
                nc.sync.dma_start(y_d.ap()[:, c * FD:c * FD + N0],
                                  yst[:, c * FD:c * FD + N0])
                nc.vector.tensor_copy(yst[:, c * FD + N0:(c + 1) * FD],
                                      accs[c][1][:])
                nc.scalar.dma_start(y_d.ap()[:, c * FD + N0:(c + 1) * FD],
                                    yst[:, c * FD + N0:(c + 1) * FD])

    nc.compile()
    return nc


def get_nc(taps=NTAPS):
    if taps not in _CACHE:
        _CACHE[taps] = _build_nc(taps)
    return _CACHE[taps]


def _prep_inputs(x, k, padding, padding_value):
    """Host-side prep: pad x, build bf16 slabs + per-core shards."""
    x = np.asarray(x, dtype=np.float32)
    k = np.asarray(k, dtype=np.float32)
    pad = bool(int(np.asarray(padding)))
    pv = float(np.asarray(padding_value))

    if pad:
        assert x.shape == (1, C, H, W), x.shape
        xp = np.full((C, H + 2 * HALF, W + 2 * HALF + 1), 0.0, dtype=np.float32)
        xp[:, :, :W + 2 * HALF] = pv
        xp[:, HALF:HALF + H, HALF:HALF + W] = x[0]
    else:
        assert x.shape == (1, C, H + 2 * HALF, W + 2 * HALF), x.shape
        xp = np.zeros((C, H + 2 * HALF, W + 2 * HALF + 1), dtype=np.float32)
        xp[:, :, :W + 2 * HALF] = x[0]

    assert k.shape == (1, NTAPS, H, W), k.shape
    # partition-block-major, tap-permuted k: [core, p, t, (r w)], bf16
    kt_all = np.ascontiguousarray(
        k[0][TAP_PERM].astype(BF16).reshape(NTAPS, NCORES, HS, NP, CPP)
        .transpose(1, 3, 0, 2, 4)).reshape(NCORES, NP, NTAPS, FD)

    cols_idx = CPP * np.arange(NP)[:, None] + np.arange(COLS_ST)[None, :]
    ident = np.eye(NP, dtype=BF16)
    in_maps = []
    for ci in range(NCORES):
        rows = slice(HS * ci, HS * ci + ROWS_ST)
        xs = np.empty((2, NP, SLABF), dtype=BF16)
        for v in (0, 1):
            sv = xp[:, rows, v:v + W + 2 * HALF]           # [C, 100, 1290]
            win = sv[:, :, cols_idx]                       # [C, 100, 128, 20]
            xs[v] = win.transpose(2, 0, 1, 3).reshape(NP, SLABF).astype(BF16)
        in_maps.append({"k": kt_all[ci], "xs": xs, "ident": ident})
    return in_maps


def _assemble_y(results):
    """results[ci]["y"] is [128, 2700] bf16; reassemble to [1, C, H, W] f32."""
    y = np.empty((C, H, W), dtype=np.float32)
    for ci in range(NCORES):
        blk = np.asarray(results[ci]["y"], dtype=np.float32)
        blk = blk.reshape(NP, C, HS, CPP)                  # [p, c, r, w]
        y[:, HS * ci:HS * (ci + 1), :] = (
            blk.transpose(1, 2, 0, 3).reshape(C, HS, W))
    return y[None]


def kernel(x, k, padding, padding_value):
    in_maps = _prep_inputs(x, k, padding, padding_value)
    nc = get_nc()
    res = run_bass_kernel_spmd(nc, in_maps, core_ids=list(range(NCORES)))
    return _assemble_y(res.results).astype(np.float32)


# revision 21
# speedup vs baseline: 1.4299x; 1.0024x over previous
"""Trainium2 Bass kernel for per-pixel kernel application (KPN-style ApplyKernel).

y[c,h,w] = sum_{ii,jj} xpad[c, h+ii, w+jj] * k[ii*11+jj, h, w]

Strategy (8 NeuronCores, data-parallel over H strips of 90 rows):
  - Partition p owns a 10-column block of W (128 partitions x 10 = 1280), with
    the +-5 column halo stored in the free dim, so both row and column shifts
    of a tap are plain access-pattern offsets (DVE lanes are partition-locked,
    so shifts must live in the free dim). All 128 lanes are used.
  - Host: pad x and build bf16 slabs [128, 3ch x 100rows x 20cols] in two
    column-alignment variants so every tap's VectorE read stays 4-byte
    aligned, keeping tensor_tensor in its 2x bf16 mode. k is re-laid-out
    host-side to bf16 [128, 121, 900] (partition-block-major, even-column
    taps first) halving HBM traffic vs f32.
  - Taps are processed in 23 runs: for each row-shift ii, the 6 even-jj (or
    5 odd-jj) taps form one run (the first ii split 2+4 so the fill-phase k
    DMA is small). Per run and channel, ONE VectorE tensor_tensor with a
    3-free-dim AP [p, tap(stride 2), row, col] computes all taps' products
    at once (bf16 2x mode, ~80ns instruction overhead amortized 6x). The
    final run falls back to per-tap ops so the PE tail stays short.
  - Products accumulate via 6 TensorE identity-matmuls per tap (K=M=128)
    into 6 PSUM banks (3 channels x 512/388-col chunks).
  - k runs are prefetched two ahead on the SP HWDGE ring; the second/third
    slab channels, ident, and the odd-alignment slab fill via the ACT ring
    in parallel.  Output y is written bf16 (host casts to f32).

  Engine budget per core (measured): DVE multiply stream ~176us (the
  bottleneck; bf16 2x mode is its ceiling), PE accumulate ~151us, DMA
  ~90us (27.9MB bf16 k + 3MB slabs + 0.7MB y at ~358GB/s/core).  GpSimd
  tensor_tensor offload was tried and rejected: a running Pool op slows
  concurrent DVE tensor_tensors ~4x (shared SBUF path), a net loss.
"""

import sys

if "/opt/trn_rl_repo" not in sys.path:
    sys.path.insert(0, "/opt/trn_rl_repo")

import numpy as np
import ml_dtypes

import concourse.mybir as mybir
from concourse import bacc
from concourse.tile import TileContext
from concourse.ap import AP
from concourse.bass_utils import run_bass_kernel_spmd

KS = 11
HALF = 5
H, W, C = 720, 1280, 3
NCORES = 8
HS = H // NCORES            # 90 rows per core
NP = 128                    # partitions (one 10-col block each)
CPP = W // NP               # 10 output cols per partition
ROWS_ST = HS + 2 * HALF     # 100 rows stored per partition
COLS_ST = CPP + 2 * HALF    # 20 cols stored per partition
SLABF = C * ROWS_ST * COLS_ST   # 6000 bf16 per partition per variant
NTAPS = KS * KS             # 121
FD = HS * CPP               # 900 elements per channel per tap
PFD = C * FD                # 2700 product elements per tap
N0, N1 = 512, FD - 512      # matmul chunk widths per channel (512, 388)

# tap order: even-jj taps first (only need slab variant 0), then odd-jj;
# within each half, ii-major so each run of same-ii taps is contiguous
TAP_PERM = ([t for t in range(NTAPS) if (t % KS) % 2 == 0]
            + [t for t in range(NTAPS) if (t % KS) % 2 == 1])
# runs of (start, ntaps): first run split 3+3 so the fill-phase k DMA is
# small, then 10 x 6 even-jj, then 11 x 5 odd-jj
RUNS = ([(0, 2), (2, 4)] + [(6 * i, 6) for i in range(1, 11)]
        + [(66 + 5 * i, 5) for i in range(11)])
CF = ROWS_ST * COLS_ST      # 2000 elements per channel slab chunk

BF16 = ml_dtypes.bfloat16

_CACHE = {}


def _build_nc(taps=NTAPS):
    assert taps == NTAPS
    nc = bacc.Bacc("TRN2", target_bir_lowering=False, debug=False)
    k_d = nc.dram_tensor("k", [NP, NTAPS, FD], mybir.dt.bfloat16, kind="ExternalInput")
    xs_d = nc.dram_tensor("xs", [2, NP, SLABF], mybir.dt.bfloat16, kind="ExternalInput")
    id_d = nc.dram_tensor("ident", [NP, NP], mybir.dt.bfloat16, kind="ExternalInput")
    y_d = nc.dram_tensor("y", [NP, PFD], mybir.dt.bfloat16, kind="ExternalOutput")

    with TileContext(nc) as tc:
        with tc.tile_pool(name="const", bufs=1) as const_pool, \
             tc.tile_pool(name="kbf", bufs=4) as kb_pool, \
             tc.tile_pool(name="prod", bufs=2) as prod_pool, \
             tc.tile_pool(name="out", bufs=1) as out_pool, \
             tc.tile_pool(name="psum", bufs=1, space="PSUM") as psum_pool:

            # per-(variant, channel) slab tiles so the first multiply only
            # waits on its own channel's DMA
            slabs = [[const_pool.tile([NP, CF], mybir.dt.bfloat16,
                                      name=f"slab{v}c{c}")
                      for c in range(C)] for v in range(2)]
            ident = const_pool.tile([NP, NP], mybir.dt.bfloat16)

            kb_tiles = {}

            def dma_krun(r, eng):
                gi0, nt = RUNS[r]
                kb = kb_pool.tile([NP, nt * FD], mybir.dt.bfloat16, name="kb")
                eng.dma_start(
                    kb[:].rearrange("p (t f) -> p t f", t=nt),
                    k_d.ap()[:, gi0:gi0 + nt, :])
                kb_tiles[r] = kb

            # Fill: the critical path (ident, slab0-c0, k run 0) goes first
            # on the SP ring (it starts ~2.5us earlier than ACT); slab1 on
            # the ACT ring in parallel.
            xsv = xs_d.ap().rearrange("v p (c f) -> v p c f", c=C)
            dma_krun(0, nc.sync)
            nc.sync.dma_start(slabs[0][0][:], xsv[0, :, 0])
            dma_krun(1, nc.sync)
            dma_krun(2, nc.sync)
            nc.scalar.dma_start(slabs[0][1][:], xsv[0, :, 1])
            nc.scalar.dma_start(slabs[0][2][:], xsv[0, :, 2])
            nc.scalar.dma_start(ident[:], id_d.ap())
            for c in range(C):
                nc.scalar.dma_start(slabs[1][c][:], xsv[1, :, c])

            slab_views = [
                [slabs[v][c][:].rearrange("p (r w) -> p r w", r=ROWS_ST)
                 for c in range(C)] for v in range(2)]

            accs = []
            for c in range(C):
                a0 = psum_pool.tile([NP, N0], mybir.dt.float32, name=f"acc{c}0")
                a1 = psum_pool.tile([NP, N1], mybir.dt.float32, name=f"acc{c}1")
                accs.append((a0, a1))

            nrun = len(RUNS)
            for r in range(nrun):
                gi0, nt = RUNS[r]
                if r + 2 < nrun and r >= 1:
                    dma_krun(r + 2, nc.sync)
                kb = kb_tiles[r]
                t0 = TAP_PERM[gi0]
                ii = t0 // KS
                v = (t0 % KS) & 1

                prod = prod_pool.tile([NP, nt * PFD], mybir.dt.bfloat16,
                                      name="prod")
                prod_t = prod[:].rearrange("p (t f) -> p t f", t=nt)

                last_run = (r == nrun - 1)
                if not last_run:
                    # one 3-free-dim TT per channel covering all nt taps:
                    # x: [p, t(stride 2 cols), r, w]; k: [p, t(stride FD), r, w]
                    jb = (TAP_PERM[gi0] % KS) - v   # first tap's jj2
                    for c in range(C):
                        base = slab_views[v][c][:, ii:ii + HS, jb:jb + CPP]
                        xs_op = AP(base.tensor, base.offset,
                                   [list(base.ap[0])] + [[2, nt]]
                                   + [list(x) for x in base.ap[1:]])
                        kap = kb[:]
                        k_op = AP(kap.tensor, kap.offset,
                                  [list(kap.ap[0])]
                                  + [[FD, nt], [CPP, HS], [1, CPP]])
                        po = prod[:].rearrange(
                            "p (t c r w) -> p t c r w", t=nt, c=C, r=HS, w=CPP
                        )[:, :, c]
                        nc.vector.tensor_tensor(po, xs_op, k_op,
                                                mybir.AluOpType.mult)
                else:
                    # per-tap, per-channel ops so the final PE tail is short
                    for dt_ in range(nt):
                        t = TAP_PERM[gi0 + dt_]
                        jj2 = (t % KS) - v
                        for c in range(C):
                            xs_op = slab_views[v][c][:, ii:ii + HS,
                                                     jj2:jj2 + CPP]
                            k_op = (kb[:, dt_ * FD:(dt_ + 1) * FD]
                                    .rearrange("p (r w) -> p r w", r=HS))
                            po = prod_t[:, dt_].rearrange(
                                "p (c r w) -> p c r w", c=C, r=HS, w=CPP
                            )[:, c]
                            nc.vector.tensor_tensor(po, xs_op, k_op,
                                                    mybir.AluOpType.mult)

                for dt_ in range(nt):
                    gi = gi0 + dt_
                    first = (gi == 0)
                    last = (gi == taps - 1)
                    pt = prod_t[:, dt_]
                    for c in range(C):
                        nc.tensor.matmul(accs[c][0][:], ident[:],
                                         pt[:, c * FD:c * FD + N0],
                                         start=first, stop=last)
                        nc.tensor.matmul(accs[c][1][:], ident[:],
                                         pt[:, c * FD + N0:(c + 1) * FD],
                                         start=first, stop=last)

            yst = out_pool.tile([NP, PFD], mybir.dt.bfloat16)
            for c in range(C):
                nc.scalar.copy(yst[:, c * FD:c * FD + N0], accs[c][0][:])
                nc.sync.dma_start(y_d.ap()[:, c * FD:c * FD + N0],
                                  yst[:, c * FD:c * FD + N0])
                nc.vector.tensor_copy(yst[:, c * FD + N0:(c + 1) * FD],
                                      accs[c][1][:])
                nc.scalar.dma_start(y_d.ap()[:, c * FD + N0:(c + 1) * FD],
                                    yst[:, c * FD + N0:(c + 1) * FD])

    nc.compile()
    return nc


def get_nc(taps=NTAPS):
    if taps not in _CACHE:
        _CACHE[taps] = _build_nc(taps)
    return _CACHE[taps]


def _prep_inputs(x, k, padding, padding_value):
    """Host-side prep: pad x, build bf16 slabs + per-core shards."""
    x = np.asarray(x, dtype=np.float32)
    k = np.asarray(k, dtype=np.float32)
    pad = bool(int(np.asarray(padding)))
    pv = float(np.asarray(padding_value))

    if pad:
        assert x.shape == (1, C, H, W), x.shape
        xp = np.full((C, H + 2 * HALF, W + 2 * HALF + 1), 0.0, dtype=np.float32)
        xp[:, :, :W + 2 * HALF] = pv
        xp[:, HALF:HALF + H, HALF:HALF + W] = x[0]
    else:
        assert x.shape == (1, C, H + 2 * HALF, W + 2 * HALF), x.shape
        xp = np.zeros((C, H + 2 * HALF, W + 2 * HALF + 1), dtype=np.float32)
        xp[:, :, :W + 2 * HALF] = x[0]

    assert k.shape == (1, NTAPS, H, W), k.shape
    # partition-block-major, tap-permuted k: [core, p, t, (r w)], bf16
    kt_all = np.ascontiguousarray(
        k[0][TAP_PERM].astype(BF16).reshape(NTAPS, NCORES, HS, NP, CPP)
        .transpose(1, 3, 0, 2, 4)).reshape(NCORES, NP, NTAPS, FD)

    cols_idx = CPP * np.arange(NP)[:, None] + np.arange(COLS_ST)[None, :]
    ident = np.eye(NP, dtype=BF16)
    in_maps = []
    for ci in range(NCORES):
        rows = slice(HS * ci, HS * ci + ROWS_ST)
        xs = np.empty((2, NP, SLABF), dtype=BF16)
        for v in (0, 1):
            sv = xp[:, rows, v:v + W + 2 * HALF]           # [C, 100, 1290]
            win = sv[:, :, cols_idx]                       # [C, 100, 128, 20]
            xs[v] = win.transpose(2, 0, 1, 3).reshape(NP, SLABF).astype(BF16)
        in_maps.append({"k": kt_all[ci], "xs": xs, "ident": ident})
    return in_maps


def _assemble_y(results):
    """results[ci]["y"] is [128, 2700] bf16; reassemble to [1, C, H, W] f32."""
    y = np.empty((C, H, W), dtype=np.float32)
    for ci in range(NCORES):
        blk = np.asarray(results[ci]["y"], dtype=np.float32)
        blk = blk.reshape(NP, C, HS, CPP)                  # [p, c, r, w]
        y[:, HS * ci:HS * (ci + 1), :] = (
            blk.transpose(1, 2, 0, 3).reshape(C, HS, W))
    return y[None]


def kernel(x, k, padding, padding_value):
    in_maps = _prep_inputs(x, k, padding, padding_value)
    nc = get_nc()
    res = run_bass_kernel_spmd(nc, in_maps, core_ids=list(range(NCORES)))
    return _assemble_y(res.results).astype(np.float32)


# revision 23
# speedup vs baseline: 1.4309x; 1.0007x over previous
"""Trainium2 Bass kernel for per-pixel kernel application (KPN-style ApplyKernel).

y[c,h,w] = sum_{ii,jj} xpad[c, h+ii, w+jj] * k[ii*11+jj, h, w]

Strategy (8 NeuronCores, data-parallel over H strips of 90 rows):
  - Partition p owns a 10-column block of W (128 partitions x 10 = 1280), with
    the +-5 column halo stored in the free dim, so both row and column shifts
    of a tap are plain access-pattern offsets (DVE lanes are partition-locked,
    so shifts must live in the free dim). All 128 lanes are used.
  - Host: pad x and build bf16 slabs [128, 3ch x 100rows x 20cols] in two
    column-alignment variants so every tap's VectorE read stays 4-byte
    aligned, keeping tensor_tensor in its 2x bf16 mode. k is re-laid-out
    host-side to bf16 [128, 121, 900] (partition-block-major, even-column
    taps first) halving HBM traffic vs f32.
  - Taps are processed in 23 runs: for each row-shift ii, the 6 even-jj (or
    5 odd-jj) taps form one run (the first ii split 2+4 so the fill-phase k
    DMA is small). Per run and channel, ONE VectorE tensor_tensor with a
    3-free-dim AP [p, tap(stride 2), row, col] computes all taps' products
    at once (bf16 2x mode, ~80ns instruction overhead amortized 6x). The
    final run falls back to per-tap ops so the PE tail stays short.
  - Products accumulate via 6 TensorE identity-matmuls per tap (K=M=128)
    into 6 PSUM banks (3 channels x 512/388-col chunks).
  - k runs are prefetched two ahead on the SP HWDGE ring; the second/third
    slab channels, ident, and the odd-alignment slab fill via the ACT ring
    in parallel.  Output y is written bf16 (host casts to f32).

  Engine budget per core (measured): DVE multiply stream ~176us (the
  bottleneck; bf16 2x mode is its ceiling), PE accumulate ~151us, DMA
  ~90us (27.9MB bf16 k + 3MB slabs + 0.7MB y at ~358GB/s/core).  GpSimd
  tensor_tensor offload was tried and rejected: a running Pool op slows
  concurrent DVE tensor_tensors ~4x (shared SBUF path), a net loss.
"""

import sys

if "/opt/trn_rl_repo" not in sys.path:
    sys.path.insert(0, "/opt/trn_rl_repo")

import numpy as np
import ml_dtypes

import concourse.mybir as mybir
from concourse import bacc
from concourse.tile import TileContext
from concourse.ap import AP
from concourse.bass_utils import run_bass_kernel_spmd

KS = 11
HALF = 5
H, W, C = 720, 1280, 3
NCORES = 8
HS = H // NCORES            # 90 rows per core
NP = 128                    # partitions (one 10-col block each)
CPP = W // NP               # 10 output cols per partition
ROWS_ST = HS + 2 * HALF     # 100 rows stored per partition
COLS_ST = CPP + 2 * HALF    # 20 cols stored per partition
SLABF = C * ROWS_ST * COLS_ST   # 6000 bf16 per partition per variant
NTAPS = KS * KS             # 121
FD = HS * CPP               # 900 elements per channel per tap
PFD = C * FD                # 2700 product elements per tap
N0, N1 = 512, FD - 512      # matmul chunk widths per channel (512, 388)

# tap order: even-jj taps first (only need slab variant 0), then odd-jj;
# within each half, ii-major so each run of same-ii taps is contiguous
TAP_PERM = ([t for t in range(NTAPS) if (t % KS) % 2 == 0]
            + [t for t in range(NTAPS) if (t % KS) % 2 == 1])
# runs of (start, ntaps): first run split 3+3 so the fill-phase k DMA is
# small, then 10 x 6 even-jj, then 11 x 5 odd-jj
RUNS = ([(0, 2), (2, 4)] + [(6 * i, 6) for i in range(1, 11)]
        + [(66 + 5 * i, 5) for i in range(11)])
CF = ROWS_ST * COLS_ST      # 2000 elements per channel slab chunk

BF16 = ml_dtypes.bfloat16

_CACHE = {}


def _build_nc(taps=NTAPS):
    assert taps == NTAPS
    nc = bacc.Bacc("TRN2", target_bir_lowering=False, debug=False)
    k_d = nc.dram_tensor("k", [NP, NTAPS, FD], mybir.dt.bfloat16, kind="ExternalInput")
    xs_d = nc.dram_tensor("xs", [2, NP, SLABF], mybir.dt.bfloat16, kind="ExternalInput")
    id_d = nc.dram_tensor("ident", [NP, NP], mybir.dt.bfloat16, kind="ExternalInput")
    y_d = nc.dram_tensor("y", [NP, PFD], mybir.dt.bfloat16, kind="ExternalOutput")

    with TileContext(nc) as tc:
        with tc.tile_pool(name="const", bufs=1) as const_pool, \
             tc.tile_pool(name="kbf", bufs=4) as kb_pool, \
             tc.tile_pool(name="prod", bufs=2) as prod_pool, \
             tc.tile_pool(name="out", bufs=1) as out_pool, \
             tc.tile_pool(name="psum", bufs=1, space="PSUM") as psum_pool:

            # per-(variant, channel) slab tiles so the first multiply only
            # waits on its own channel's DMA
            slabs = [[const_pool.tile([NP, CF], mybir.dt.bfloat16,
                                      name=f"slab{v}c{c}")
                      for c in range(C)] for v in range(2)]
            ident = const_pool.tile([NP, NP], mybir.dt.bfloat16)

            kb_tiles = {}

            def dma_krun(r, eng):
                gi0, nt = RUNS[r]
                kb = kb_pool.tile([NP, nt * FD], mybir.dt.bfloat16, name="kb")
                eng.dma_start(
                    kb[:].rearrange("p (t f) -> p t f", t=nt),
                    k_d.ap()[:, gi0:gi0 + nt, :])
                kb_tiles[r] = kb

            # Fill: the critical path (ident, slab0-c0, k run 0) goes first
            # on the SP ring (it starts ~2.5us earlier than ACT); slab1 on
            # the ACT ring in parallel.
            xsv = xs_d.ap().rearrange("v p (c f) -> v p c f", c=C)
            dma_krun(0, nc.sync)
            nc.sync.dma_start(slabs[0][0][:], xsv[0, :, 0])
            dma_krun(1, nc.sync)
            dma_krun(2, nc.sync)
            nc.scalar.dma_start(slabs[0][1][:], xsv[0, :, 1])
            nc.scalar.dma_start(slabs[0][2][:], xsv[0, :, 2])
            nc.scalar.dma_start(ident[:], id_d.ap())

            slab_views = [
                [slabs[v][c][:].rearrange("p (r w) -> p r w", r=ROWS_ST)
                 for c in range(C)] for v in range(2)]

            accs = []
            for c in range(C):
                a0 = psum_pool.tile([NP, N0], mybir.dt.float32, name=f"acc{c}0")
                a1 = psum_pool.tile([NP, N1], mybir.dt.float32, name=f"acc{c}1")
                accs.append((a0, a1))

            nrun = len(RUNS)
            for r in range(nrun):
                gi0, nt = RUNS[r]
                if r + 2 < nrun and r >= 1:
                    dma_krun(r + 2, nc.sync)
                if r == 2:
                    # odd-alignment slab (first needed at run 12, ~95us in):
                    # deferred past the fill crunch so it doesn't steal
                    # bandwidth from the critical-path k/slab0 DMAs
                    for c in range(C):
                        nc.scalar.dma_start(slabs[1][c][:], xsv[1, :, c])
                kb = kb_tiles[r]
                t0 = TAP_PERM[gi0]
                ii = t0 // KS
                v = (t0 % KS) & 1

                prod = prod_pool.tile([NP, nt * PFD], mybir.dt.bfloat16,
                                      name="prod")
                prod_t = prod[:].rearrange("p (t f) -> p t f", t=nt)

                last_run = (r == nrun - 1)
                if not last_run:
                    # one 3-free-dim TT per channel covering all nt taps:
                    # x: [p, t(stride 2 cols), r, w]; k: [p, t(stride FD), r, w]
                    jb = (TAP_PERM[gi0] % KS) - v   # first tap's jj2
                    for c in range(C):
                        base = slab_views[v][c][:, ii:ii + HS, jb:jb + CPP]
                        xs_op = AP(base.tensor, base.offset,
                                   [list(base.ap[0])] + [[2, nt]]
                                   + [list(x) for x in base.ap[1:]])
                        kap = kb[:]
                        k_op = AP(kap.tensor, kap.offset,
                                  [list(kap.ap[0])]
                                  + [[FD, nt], [CPP, HS], [1, CPP]])
                        po = prod[:].rearrange(
                            "p (t c r w) -> p t c r w", t=nt, c=C, r=HS, w=CPP
                        )[:, :, c]
                        nc.vector.tensor_tensor(po, xs_op, k_op,
                                                mybir.AluOpType.mult)
                else:
                    # per-tap, per-channel ops so the final PE tail is short
                    for dt_ in range(nt):
                        t = TAP_PERM[gi0 + dt_]
                        jj2 = (t % KS) - v
                        for c in range(C):
                            xs_op = slab_views[v][c][:, ii:ii + HS,
                                                     jj2:jj2 + CPP]
                            k_op = (kb[:, dt_ * FD:(dt_ + 1) * FD]
                                    .rearrange("p (r w) -> p r w", r=HS))
                            po = prod_t[:, dt_].rearrange(
                                "p (c r w) -> p c r w", c=C, r=HS, w=CPP
                            )[:, c]
                            nc.vector.tensor_tensor(po, xs_op, k_op,
                                                    mybir.AluOpType.mult)

                for dt_ in range(nt):
                    gi = gi0 + dt_
                    first = (gi == 0)
                    last = (gi == taps - 1)
                    pt = prod_t[:, dt_]
                    for c in range(C):
                        nc.tensor.matmul(accs[c][0][:], ident[:],
                                         pt[:, c * FD:c * FD + N0],
                                         start=first, stop=last)
                        nc.tensor.matmul(accs[c][1][:], ident[:],
                                         pt[:, c * FD + N0:(c + 1) * FD],
                                         start=first, stop=last)

            yst = out_pool.tile([NP, PFD], mybir.dt.bfloat16)
            for c in range(C):
                nc.scalar.copy(yst[:, c * FD:c * FD + N0], accs[c][0][:])
                nc.sync.dma_start(y_d.ap()[:, c * FD:c * FD + N0],
                                  yst[:, c * FD:c * FD + N0])
                nc.vector.tensor_copy(yst[:, c * FD + N0:(c + 1) * FD],
                                      accs[c][1][:])
                nc.scalar.dma_start(y_d.ap()[:, c * FD + N0:(c + 1) * FD],
                                    yst[:, c * FD + N0:(c + 1) * FD])

    nc.compile()
    return nc


def get_nc(taps=NTAPS):
    if taps not in _CACHE:
        _CACHE[taps] = _build_nc(taps)
    return _CACHE[taps]


def _prep_inputs(x, k, padding, padding_value):
    """Host-side prep: pad x, build bf16 slabs + per-core shards."""
    x = np.asarray(x, dtype=np.float32)
    k = np.asarray(k, dtype=np.float32)
    pad = bool(int(np.asarray(padding)))
    pv = float(np.asarray(padding_value))

    if pad:
        assert x.shape == (1, C, H, W), x.shape
        xp = np.full((C, H + 2 * HALF, W + 2 * HALF + 1), 0.0, dtype=np.float32)
        xp[:, :, :W + 2 * HALF] = pv
        xp[:, HALF:HALF + H, HALF:HALF + W] = x[0]
    else:
        assert x.shape == (1, C, H + 2 * HALF, W + 2 * HALF), x.shape
        xp = np.zeros((C, H + 2 * HALF, W + 2 * HALF + 1), dtype=np.float32)
        xp[:, :, :W + 2 * HALF] = x[0]

    assert k.shape == (1, NTAPS, H, W), k.shape
    # partition-block-major, tap-permuted k: [core, p, t, (r w)], bf16
    kt_all = np.ascontiguousarray(
        k[0][TAP_PERM].astype(BF16).reshape(NTAPS, NCORES, HS, NP, CPP)
        .transpose(1, 3, 0, 2, 4)).reshape(NCORES, NP, NTAPS, FD)

    cols_idx = CPP * np.arange(NP)[:, None] + np.arange(COLS_ST)[None, :]
    ident = np.eye(NP, dtype=BF16)
    in_maps = []
    for ci in range(NCORES):
        rows = slice(HS * ci, HS * ci + ROWS_ST)
        xs = np.empty((2, NP, SLABF), dtype=BF16)
        for v in (0, 1):
            sv = xp[:, rows, v:v + W + 2 * HALF]           # [C, 100, 1290]
            win = sv[:, :, cols_idx]                       # [C, 100, 128, 20]
            xs[v] = win.transpose(2, 0, 1, 3).reshape(NP, SLABF).astype(BF16)
        in_maps.append({"k": kt_all[ci], "xs": xs, "ident": ident})
    return in_maps


def _assemble_y(results):
    """results[ci]["y"] is [128, 2700] bf16; reassemble to [1, C, H, W] f32."""
    y = np.empty((C, H, W), dtype=np.float32)
    for ci in range(NCORES):
        blk = np.asarray(results[ci]["y"], dtype=np.float32)
        blk = blk.reshape(NP, C, HS, CPP)                  # [p, c, r, w]
        y[:, HS * ci:HS * (ci + 1), :] = (
            blk.transpose(1, 2, 0, 3).reshape(C, HS, W))
    return y[None]


def kernel(x, k, padding, padding_value):
    in_maps = _prep_inputs(x, k, padding, padding_value)
    nc = get_nc()
    res = run_bass_kernel_spmd(nc, in_maps, core_ids=list(range(NCORES)))
    return _assemble_y(res.results).astype(np.float32)
